# revision 1
# baseline (speedup 1.0000x reference)
"""Fused multi-head attention (B=2, T=2048, D=2048, H=16) on 8 trn2 NeuronCores.

Sharding: core c handles batch b=c//4 and heads [4g, 4g+4), g=c%4 (tensor
parallel over heads x data parallel over batch). Each core computes its
4 heads' contribution to out[b] = attn(x[b]) @ Wo^T; the host sums the 4
partials per batch.

Device algorithm (all matmuls fp32r, PSUM fp32):
  P1a  qT = (Wq_s/sqrt(dh)) @ x^T, kT = Wk_s @ x^T       [E=512, T]
  P1b  v  = x @ Wv_s^T                                    [T, E]
  P2   per i-chunk (512 queries), per head:
         S^T[j,i] = kT_h^T-contract : PSUM -> exp (ACT) -> * exp(mask^T)
         ctx^T[e,i] += v_h^T @ P^T  (PSUM, 16 j-tiles)
         l[i]      += 1^T @ P^T     (PSUM)
         ctx^T *= broadcast(1/l)    (outer-product bcast + DVE mul)
  P3   out[t,dd] = sum_e ctx^T[e,t] * WoT[e,dd]  -> DRAM

Inputs are pre-transposed/sharded/scaled on the host so every device matmul
is a natural [K=128-tile] x [N<=512] fp32r op.
"""

import os

import numpy as np

import concourse.bass as bass
import concourse.mybir as mybir
import concourse.tile as tile
from concourse import bacc
from concourse.bass_utils import run_bass_kernel_spmd

F32 = mybir.dt.float32
F32R = mybir.dt.float32r
EXP = mybir.ActivationFunctionType.Exp

B, T, D, H = 2, 2048, 2048, 16
DH = D // H          # 128
E = 512              # features per core (4 heads)
HPC = 4              # heads per core
NT = T // 128        # 16 token tiles
ND = D // 128        # 16 model-dim tiles
NE = E // 128        # 4 e-tiles per core
NI = T // 512        # 4 i-chunks (moving dim)
NJ = NT              # 16 j-tiles

_NC_CACHE = {}

# per-(jt, ic) mask-block class: 0 = fully masked (skip), 1 = unmasked
# (skip the mask multiply), 2 = mixed (apply exp(mask) elementwise)
SKIP, NOMULT, MIXED = 0, 1, 2


def _build(cls_key):
    cls = np.asarray(cls_key, dtype=np.int64).reshape(NJ, NI)
    nc = bacc.Bacc(None, target_bir_lowering=False, debug=False)
    xt = nc.declare_dram_parameter("xt", [D, T], F32R, isOutput=False)
    wq = nc.declare_dram_parameter("wq", [D, E], F32R, isOutput=False)
    wk = nc.declare_dram_parameter("wk", [D, E], F32R, isOutput=False)
    wv = nc.declare_dram_parameter("wv", [D, E], F32R, isOutput=False)
    wo = nc.declare_dram_parameter("wo", [E, D], F32R, isOutput=False)
    em = nc.declare_dram_parameter("em", [T, T], F32, isOutput=False)
    onk = nc.declare_dram_parameter("onk", [128, 1], F32R, isOutput=False)
    onp = nc.declare_dram_parameter("onp", [1, 128], F32R, isOutput=False)
    idn = nc.declare_dram_parameter("idn", [128, 128], F32R, isOutput=False)
    out = nc.declare_dram_parameter("out", [T, D], F32, isOutput=True)

    with tile.TileContext(nc) as tc:
        # ---- long-lived residents (stack order: ctx outlives qk/v) -----
        pool_ctx = tc.alloc_tile_pool(name="res_ctx", bufs=1)
        ctx = [pool_ctx.tile([128, T], F32R, name=f"ctx{m}") for m in range(NE)]
        pool_v = tc.alloc_tile_pool(name="res_v", bufs=1)
        v_sb = pool_v.tile([128, NT, E], F32R)
        pool_qk = tc.alloc_tile_pool(name="res_qk", bufs=1)
        qT = [pool_qk.tile([128, T], F32R, name=f"qT{m}") for m in range(NE)]
        kT = [pool_qk.tile([128, T], F32R, name=f"kT{m}") for m in range(NE)]

        scope_p1a = nc.named_scope("P1a_qk"); scope_p1a.__enter__()
        # ---- P1a: q/k projections --------------------------------------
        p_w = tc.alloc_tile_pool(name="p1w", bufs=1)
        wq_sb = p_w.tile([128, ND, E], F32R)
        wk_sb = p_w.tile([128, ND, E], F32R)
        for dt in range(ND):
            nc.sync.dma_start(out=wq_sb[:, dt, :], in_=wq.ap()[dt * 128:(dt + 1) * 128, :])
            nc.sync.dma_start(out=wk_sb[:, dt, :], in_=wk.ap()[dt * 128:(dt + 1) * 128, :])
        p_x = tc.alloc_tile_pool(name="p1x", bufs=3)
        p_ps1 = tc.alloc_tile_pool(name="p1ps", bufs=8, space="PSUM")
        for nch in range(NI):
            psq, psk = {}, {}
            for m in range(NE):
                ps_q = p_ps1.tile([128, 512], F32, name="ps_q", bufs=4)
                ps_k = p_ps1.tile([128, 512], F32, name="ps_k", bufs=4)
                psq[m], psk[m] = ps_q, ps_k
            for dt in range(ND):
                xtile = p_x.tile([128, 512], F32R, name="xtile")
                nc.sync.dma_start(
                    out=xtile,
                    in_=xt.ap()[dt * 128:(dt + 1) * 128, nch * 512:(nch + 1) * 512])
                st, sp = dt == 0, dt == ND - 1
                for m in range(NE):
                    nc.tensor.matmul(psq[m], wq_sb[:, dt, m * 128:(m + 1) * 128],
                                     xtile, start=st, stop=sp)
                    nc.tensor.matmul(psk[m], wk_sb[:, dt, m * 128:(m + 1) * 128],
                                     xtile, start=st, stop=sp)
            for m in range(NE):
                nc.scalar.copy(qT[m][:, nch * 512:(nch + 1) * 512], psq[m])
                nc.vector.tensor_copy(kT[m][:, nch * 512:(nch + 1) * 512], psk[m])
        p_ps1.release()
        p_x.release()
        p_w.release()
        scope_p1a.__exit__(None, None, None)
        scope_p1b = nc.named_scope("P1b_v"); scope_p1b.__enter__()

        # ---- P1b: v via vT = Wv_s @ x^T, then PE-transpose -------------
        p_wv = tc.alloc_tile_pool(name="p1bw", bufs=1)
        wv_sb = p_wv.tile([128, ND, E], F32R)
        idn_sb = p_wv.tile([128, 128], F32R)
        nc.sync.dma_start(out=idn_sb, in_=idn.ap())
        for dt in range(ND):
            nc.sync.dma_start(out=wv_sb[:, dt, :], in_=wv.ap()[dt * 128:(dt + 1) * 128, :])
        p_vt = tc.alloc_tile_pool(name="p1bvt", bufs=2)
        p_x2 = tc.alloc_tile_pool(name="p1bx", bufs=3)
        p_ps2 = tc.alloc_tile_pool(name="p1bps", bufs=4, space="PSUM")
        p_pst = tc.alloc_tile_pool(name="p1bpst", bufs=4, space="PSUM")
        for nch in range(NI):
            psv = {}
            for m in range(NE):
                ps_v = p_ps2.tile([128, 512], F32, name="ps_v", bufs=4)
                psv[m] = ps_v
            for dt in range(ND):
                xtile2 = p_x2.tile([128, 512], F32R, name="xtile2")
                nc.sync.dma_start(
                    out=xtile2,
                    in_=xt.ap()[dt * 128:(dt + 1) * 128, nch * 512:(nch + 1) * 512])
                for m in range(NE):
                    nc.tensor.matmul(psv[m], wv_sb[:, dt, m * 128:(m + 1) * 128],
                                     xtile2, start=(dt == 0), stop=(dt == ND - 1))
            vtc = p_vt.tile([128, NE, 512], F32R, name="vtc")
            for m in range(NE):
                nc.scalar.copy(vtc[:, m, :], psv[m])
            for m in range(NE):
                for tl in range(4):
                    ps_t = p_pst.tile([128, 128], F32R, name="ps_t")
                    nc.tensor.transpose(
                        ps_t, vtc[:, m, tl * 128:(tl + 1) * 128], idn_sb)
                    nc.vector.tensor_copy(
                        v_sb[:, nch * 4 + tl, m * 128:(m + 1) * 128], ps_t)
        p_pst.release()
        p_ps2.release()
        p_x2.release()
        p_vt.release()
        p_wv.release()
        scope_p1b.__exit__(None, None, None)
        scope_p2 = nc.named_scope("P2_attn"); scope_p2.__enter__()

        # ---- P2: attention ---------------------------------------------
        p_const = tc.alloc_tile_pool(name="p2c", bufs=1)
        ones_k = p_const.tile([128, 1], F32R)
        ones_p = p_const.tile([1, 128], F32R)
        nc.sync.dma_start(out=ones_k, in_=onk.ap())
        nc.sync.dma_start(out=ones_p, in_=onp.ap())

        p_em = tc.alloc_tile_pool(name="p2em", bufs=3)
        p_pt = tc.alloc_tile_pool(name="p2pt", bufs=3)
        p_ptm = tc.alloc_tile_pool(name="p2ptm", bufs=3)
        p_bs = tc.alloc_tile_pool(name="p2bs", bufs=2)
        p_rr = tc.alloc_tile_pool(name="p2rr", bufs=2)
        ps_ctx_pool = tc.alloc_tile_pool(name="p2psc", bufs=2, space="PSUM")
        ps_l_pool = tc.alloc_tile_pool(name="p2psl", bufs=2, space="PSUM")
        ps_s_pool = tc.alloc_tile_pool(name="p2pss", bufs=3, space="PSUM")
        ps_b_pool = tc.alloc_tile_pool(name="p2psb", bufs=1, space="PSUM")

        for ic in range(NI):
            isl = slice(ic * 512, (ic + 1) * 512)
            surv = [jt for jt in range(NJ) if cls[jt, ic] != SKIP]
            assert surv, f"i-chunk {ic}: every key block masked"
            first, last = surv[0], surv[-1]
            for hp in range(HPC // 2):
                heads = (2 * hp, 2 * hp + 1)
                cps, lps = {}, {}
                for h in heads:
                    ps_c = ps_ctx_pool.tile([128, 512], F32, name="ps_c")
                    ps_l = ps_l_pool.tile([1, 512], F32, name="ps_l")
                    cps[h], lps[h] = ps_c, ps_l
                for jt in surv:
                    if cls[jt, ic] == MIXED:
                        emt = p_em.tile([128, 512], F32, name="emt")
                        nc.sync.dma_start(
                            out=emt, in_=em.ap()[jt * 128:(jt + 1) * 128, isl])
                    for h in heads:
                        ps_s = ps_s_pool.tile([128, 512], F32, name="ps_s")
                        nc.tensor.matmul(
                            ps_s, kT[h][:, jt * 128:(jt + 1) * 128],
                            qT[h][:, isl], start=True, stop=True)
                        pt = p_pt.tile([128, 512], F32R, name="pt")
                        nc.scalar.activation(pt, ps_s, EXP)
                        if cls[jt, ic] == MIXED:
                            ptm = p_ptm.tile([128, 512], F32R, name="ptm")
                            nc.vector.tensor_mul(ptm, pt, emt)
                        else:
                            ptm = pt
                        st, sp = jt == first, jt == last
                        nc.tensor.matmul(
                            cps[h], v_sb[:, jt, h * 128:(h + 1) * 128],
                            ptm, start=st, stop=sp)
                        nc.tensor.matmul(lps[h], ones_k, ptm,
                                         start=st, stop=sp)
                for h in heads:
                    rr = p_rr.tile([1, 512], F32R, name="rr")
                    with nc.allow_low_precision(reason="softmax recip f32r"):
                        nc.vector.reciprocal(rr, lps[h])
                    ps_b = ps_b_pool.tile([128, 512], F32, name="ps_b")
                    nc.tensor.matmul(ps_b, ones_p, rr, start=True, stop=True)
                    bsb = p_bs.tile([128, 512], F32, name="bsb")
                    nc.scalar.copy(bsb, ps_b)
                    nc.vector.tensor_mul(ctx[h][:, isl], cps[h], bsb)
        for p in (ps_b_pool, ps_s_pool, ps_l_pool, ps_ctx_pool,
                  p_rr, p_bs, p_ptm, p_pt, p_em, p_const):
            p.release()
        pool_qk.release()
        pool_v.release()
        scope_p2.__exit__(None, None, None)
        scope_p3 = nc.named_scope("P3_out"); scope_p3.__enter__()

        # ---- P3: output projection -------------------------------------
        p_wo = tc.alloc_tile_pool(name="p3w", bufs=1)
        wo_sb = p_wo.tile([128, NE, D], F32R)
        for et in range(NE):
            nc.sync.dma_start(out=wo_sb[:, et, :], in_=wo.ap()[et * 128:(et + 1) * 128, :])
        p_ot = tc.alloc_tile_pool(name="p3o", bufs=3)
        p_ps3 = tc.alloc_tile_pool(name="p3ps", bufs=3, space="PSUM")
        for tt in range(NT):
            tsl = slice(tt * 128, (tt + 1) * 128)
            for nch in range(NI):
                ps_o = p_ps3.tile([128, 512], F32, name="ps_o")
                for et in range(NE):
                    nc.tensor.matmul(
                        ps_o, ctx[et][:, tsl],
                        wo_sb[:, et, nch * 512:(nch + 1) * 512],
                        start=(et == 0), stop=(et == NE - 1))
                ot = p_ot.tile([128, 512], F32, name="ot")
                nc.scalar.copy(ot, ps_o)
                nc.sync.dma_start(
                    out=out.ap()[tsl, nch * 512:(nch + 1) * 512], in_=ot)
        p_ps3.release()
        p_ot.release()
        p_wo.release()
        pool_ctx.release()
        scope_p3.__exit__(None, None, None)

    nc.compile()
    return nc


def _get_nc(cls_key):
    if cls_key not in _NC_CACHE:
        _NC_CACHE[cls_key] = _build(cls_key)
    return _NC_CACHE[cls_key]


def kernel(x, Wq, Wk, Wv, Wo, attn_mask):
    x = np.asarray(x, dtype=np.float32)
    Wq = np.asarray(Wq, dtype=np.float32)
    Wk = np.asarray(Wk, dtype=np.float32)
    Wv = np.asarray(Wv, dtype=np.float32)
    Wo = np.asarray(Wo, dtype=np.float32)
    mask = np.asarray(attn_mask, dtype=np.float32).reshape(T, T)

    emT = np.ascontiguousarray(np.exp(mask).T)
    xT = [np.ascontiguousarray(x[b].T) for b in range(B)]
    scale = np.float32(1.0 / np.sqrt(DH))

    blocks = emT.reshape(NJ, 128, NI, 512)
    cls = np.full((NJ, NI), MIXED, dtype=np.int64)
    for jt in range(NJ):
        for ic in range(NI):
            sub = blocks[jt, :, ic, :]
            if not sub.any():
                cls[jt, ic] = SKIP
            elif np.all(sub == 1.0):
                cls[jt, ic] = NOMULT
    cls_key = tuple(cls.flatten().tolist())

    in_maps = []
    for c in range(8):
        b, g = c // 4, c % 4
        rows = slice(E * g, E * (g + 1))
        in_maps.append({
            "xt": xT[b],
            "wq": np.ascontiguousarray((Wq[rows, :] * scale).T),
            "wk": np.ascontiguousarray(Wk[rows, :].T),
            "wv": np.ascontiguousarray(Wv[rows, :].T),
            "wo": np.ascontiguousarray(Wo[:, rows].T),
            "em": emT,
            "onk": np.ones((128, 1), dtype=np.float32),
            "onp": np.ones((1, 128), dtype=np.float32),
            "idn": np.eye(128, dtype=np.float32),
        })

    global _LAST_IN_MAPS, _LAST_NC
    _LAST_IN_MAPS = in_maps
    nc = _get_nc(cls_key)
    _LAST_NC = nc
    res = run_bass_kernel_spmd(nc, in_maps, list(range(8)))
    outs = [r["out"] for r in res.results]
    full = np.stack([
        outs[0] + outs[1] + outs[2] + outs[3],
        outs[4] + outs[5] + outs[6] + outs[7],
    ]).astype(np.float32)
    return full



# revision 2
# speedup vs baseline: 1.1977x; 1.1977x over previous
"""Fused multi-head attention (B=2, T=2048, D=2048, H=16) on 8 trn2 NeuronCores.

Sharding: core c handles batch b=c//4 and heads [4g, 4g+4), g=c%4 (tensor
parallel over heads x data parallel over batch). Each core computes its
4 heads' contribution to out[b] = attn(x[b]) @ Wo^T; the host sums the 4
partials per batch.

v2: single fused loop over 512-token chunks (causality: chunk ic's attention
only needs K/V from chunks <= ic), all-bf16 matmul operands (f32 PSUM),
V projected directly into [token, feature] layout (no PE transposes),
diagonal attention blocks computed at partial width with one shared
128x128 triangular mask constant.

Per chunk ic (tokens [512*ic, 512*ic+512)):
  P1  qT[m][:, chunk] = (Wq_s/sqrt(dh) @ x^T)   per m (4 feature tiles)
      kT[m][:, chunk] =  Wk_s @ x^T
      v[4ic+jl]       =  x-block^T-stationary @ Wv  -> [tok, feat]
  P2  per head h: for each surviving key block jt (descending col offset):
        S^T = kT-block^T-contract @ qT[:, off:]  (PSUM)
        pt  = exp(S^T)  (ACT, bf16)   [triangular sub-block *= tri]
        ctx^T[:, off:] += v-block^T @ pt ; l[off:] += 1^T @ pt
      ctx[h][:, chunk] = cps * broadcast(1/l)
  P3  out[t-block, :] = sum_e ctx^T[e, t-block] @ Wo -> DRAM (f32)
"""

import numpy as np
import ml_dtypes

import concourse.bass as bass
import concourse.mybir as mybir
import concourse.tile as tile
from concourse import bacc
from concourse.bass_utils import run_bass_kernel_spmd

F32 = mybir.dt.float32
F32R = mybir.dt.float32r
BF16 = mybir.dt.bfloat16
EXP = mybir.ActivationFunctionType.Exp
BF = ml_dtypes.bfloat16

B, T, D, H = 2, 2048, 2048, 16
DH = D // H          # 128
E = 512              # features per core (4 heads)
HPC = 4              # heads per core
NT = T // 128        # 16 token tiles
ND = D // 128        # 16 model-dim tiles
NE = E // 128        # 4 e-tiles per core
NI = T // 512        # 4 token chunks
NJ = NT              # 16 key tiles

_NC_CACHE = {}


def _build(blocks_key):
    # blocks_key: tuple over ic of tuple of (jt, off, mixed_tuple) where
    # mixed_tuple is ((c, pat_idx), ...) for 128-col sub-blocks needing an
    # elementwise mask multiply; n_pat = number of distinct mask patterns.
    blocks_per_ic, n_pat = blocks_key
    nc = bacc.Bacc(None, target_bir_lowering=False, debug=False)
    xt = nc.declare_dram_parameter("xt", [D, T], BF16, isOutput=False)
    wq = nc.declare_dram_parameter("wq", [D, E], BF16, isOutput=False)
    wk = nc.declare_dram_parameter("wk", [D, E], BF16, isOutput=False)
    wv = nc.declare_dram_parameter("wv", [D, E], BF16, isOutput=False)
    wo = nc.declare_dram_parameter("wo", [E, D], BF16, isOutput=False)
    em = nc.declare_dram_parameter("em", [128, n_pat, 128], BF16, isOutput=False)
    onk = nc.declare_dram_parameter("onk", [128, 1], BF16, isOutput=False)
    onp = nc.declare_dram_parameter("onp", [1, 128], F32R, isOutput=False)
    out = nc.declare_dram_parameter("out", [T, D], F32, isOutput=True)

    with tile.TileContext(nc) as tc:
        # ---- long-lived residents ---------------------------------------
        p_res = tc.alloc_tile_pool(name="res", bufs=1)
        qT = [p_res.tile([128, T], BF16, name=f"qT{m}") for m in range(NE)]
        kT = [p_res.tile([128, T], BF16, name=f"kT{m}") for m in range(NE)]
        ctx = [p_res.tile([128, T], BF16, name=f"ctx{m}") for m in range(NE)]
        v_sb = p_res.tile([128, NT, E], BF16)
        wq_sb = p_res.tile([128, ND, E], BF16)
        wk_sb = p_res.tile([128, ND, E], BF16)
        wv_sb = p_res.tile([128, ND, E], BF16)
        wo_sb = p_res.tile([128, NE, D], BF16)
        em_sb = p_res.tile([128, n_pat, 128], BF16)
        onk_sb = p_res.tile([128, 1], BF16)
        onp_sb = p_res.tile([1, 128], F32R)

        for dt in range(ND):
            nc.sync.dma_start(out=wq_sb[:, dt, :], in_=wq.ap()[dt * 128:(dt + 1) * 128, :])
            nc.sync.dma_start(out=wk_sb[:, dt, :], in_=wk.ap()[dt * 128:(dt + 1) * 128, :])
            nc.sync.dma_start(out=wv_sb[:, dt, :], in_=wv.ap()[dt * 128:(dt + 1) * 128, :])
        nc.sync.dma_start(out=em_sb[:, :, :], in_=em.ap())
        nc.sync.dma_start(out=onk_sb, in_=onk.ap())
        nc.sync.dma_start(out=onp_sb, in_=onp.ap())
        for et in range(NE):
            nc.sync.dma_start(out=wo_sb[:, et, :], in_=wo.ap()[et * 128:(et + 1) * 128, :])

        # ---- working pools ----------------------------------------------
        p_x = tc.alloc_tile_pool(name="px", bufs=2)
        p_pt = tc.alloc_tile_pool(name="ppt", bufs=4)
        p_ot = tc.alloc_tile_pool(name="pot", bufs=3)
        p_bs = tc.alloc_tile_pool(name="pbs", bufs=2)
        p_rr = tc.alloc_tile_pool(name="prr", bufs=2)
        ps_big = tc.alloc_tile_pool(name="psbig", bufs=4, space="PSUM")
        ps_cps = tc.alloc_tile_pool(name="pscps", bufs=2, space="PSUM")
        ps_sm = tc.alloc_tile_pool(name="pssm", bufs=1, space="PSUM")

        for ic in range(NI):
            csl = slice(ic * 512, (ic + 1) * 512)
            scope = nc.named_scope(f"chunk{ic}")
            scope.__enter__()

            # ---- P1: projections for this chunk -------------------------
            xc = p_x.tile([128, ND, 512], BF16, name="xc", bufs=2)
            for dt in range(ND):
                nc.sync.dma_start(
                    out=xc[:, dt, :], in_=xt.ap()[dt * 128:(dt + 1) * 128, csl])
            for m in range(NE):
                msl = slice(m * 128, (m + 1) * 128)
                psq = ps_big.tile([128, 512], F32, name="ps", bufs=4)
                psk = ps_big.tile([128, 512], F32, name="ps", bufs=4)
                for dt in range(ND):
                    st, sp = dt == 0, dt == ND - 1
                    nc.tensor.matmul(psq, wq_sb[:, dt, msl], xc[:, dt, :],
                                     start=st, stop=sp)
                    nc.tensor.matmul(psk, wk_sb[:, dt, msl], xc[:, dt, :],
                                     start=st, stop=sp)
                nc.scalar.copy(qT[m][:, csl], psq)
                nc.vector.tensor_copy(kT[m][:, csl], psk)
            for jl in range(4):
                jt = ic * 4 + jl
                psv = ps_big.tile([128, 512], F32, name="ps", bufs=4)
                for dt in range(ND):
                    nc.tensor.matmul(
                        psv, xc[:, dt, jl * 128:(jl + 1) * 128], wv_sb[:, dt, :],
                        start=(dt == 0), stop=(dt == ND - 1))
                nc.vector.tensor_copy(v_sb[:, jt, :], psv)

            # ---- P2: attention for this chunk ---------------------------
            blocks = blocks_per_ic[ic]
            nb = len(blocks)
            for h in range(HPC):
                hsl = slice(h * 128, (h + 1) * 128)
                cps = ps_cps.tile([128, 512], F32, name="cps", bufs=2)
                lps = ps_sm.tile([1, 512], F32, name="lps", bufs=1)
                for bi, (jt, off, mixed) in enumerate(blocks):
                    ps_s = ps_big.tile([128, 512], F32, name="ps", bufs=4)
                    nc.tensor.matmul(
                        ps_s[:, off:512], kT[h][:, jt * 128:(jt + 1) * 128],
                        qT[h][:, ic * 512 + off:(ic + 1) * 512],
                        start=True, stop=True)
                    pt = p_pt.tile([128, 512], BF16, name="pt", bufs=4)
                    nc.scalar.activation(pt[:, off:512], ps_s[:, off:512], EXP)
                    for (c, pidx) in mixed:
                        nc.vector.tensor_mul(
                            pt[:, c * 128:(c + 1) * 128],
                            pt[:, c * 128:(c + 1) * 128],
                            em_sb[:, pidx, :])
                    st, sp = bi == 0, bi == nb - 1
                    nc.tensor.matmul(cps[:, off:512], v_sb[:, jt, hsl],
                                     pt[:, off:512], start=st, stop=sp)
                    nc.tensor.matmul(lps[:, off:512], onk_sb, pt[:, off:512],
                                     start=st, stop=sp)
                rr = p_rr.tile([1, 512], F32R, name="rr", bufs=2)
                with nc.allow_low_precision(reason="softmax recip f32r"):
                    nc.vector.reciprocal(rr, lps)
                ps_b = ps_sm.tile([128, 512], F32, name="ps_b", bufs=1)
                nc.tensor.matmul(ps_b, onp_sb, rr, start=True, stop=True)
                bsb = p_bs.tile([128, 512], F32, name="bsb", bufs=2)
                nc.scalar.copy(bsb, ps_b)
                nc.vector.tensor_mul(ctx[h][:, csl], cps, bsb)

            # ---- P3: output projection for this chunk's tokens ----------
            for tl in range(4):
                tt = ic * 4 + tl
                tsl = slice(tt * 128, (tt + 1) * 128)
                for nch in range(NI):
                    ps_o = ps_big.tile([128, 512], F32, name="ps", bufs=4)
                    for et in range(NE):
                        nc.tensor.matmul(
                            ps_o, ctx[et][:, tsl],
                            wo_sb[:, et, nch * 512:(nch + 1) * 512],
                            start=(et == 0), stop=(et == NE - 1))
                    ot = p_ot.tile([128, 512], F32, name="ot", bufs=3)
                    if (tl + nch) % 4 == 0:
                        nc.scalar.copy(ot, ps_o)
                    else:
                        nc.vector.tensor_copy(ot, ps_o)
                    nc.sync.dma_start(
                        out=out.ap()[tsl, nch * 512:(nch + 1) * 512], in_=ot)
            scope.__exit__(None, None, None)

        for p in (ps_sm, ps_cps, ps_big, p_rr, p_bs, p_ot, p_pt, p_x, p_res):
            p.release()

    nc.compile()
    return nc


def _classify(mask):
    """Per (ic, jt): column offset + mixed 128-col sub-blocks, from exp(mask)^T."""
    emT = np.ascontiguousarray(np.exp(mask).T)  # [key j, query i]
    pats = {}   # pattern bytes -> index
    pat_list = []
    blocks_per_ic = []
    for ic in range(NI):
        blk = []
        for jt in range(NJ):
            sub = emT[jt * 128:(jt + 1) * 128, ic * 512:(ic + 1) * 512]
            # 128-col sub-block classes
            kinds = []
            for c in range(4):
                s = sub[:, c * 128:(c + 1) * 128]
                if not s.any():
                    kinds.append(0)
                elif np.all(s == 1.0):
                    kinds.append(1)
                else:
                    kinds.append(2)
            if all(k == 0 for k in kinds):
                continue
            first = next(i for i, k in enumerate(kinds) if k != 0)
            off = first * 128
            mixed = []
            for c in range(first, 4):
                if kinds[c] != 1:
                    s = np.asarray(sub[:, c * 128:(c + 1) * 128], dtype=np.float32)
                    key = s.tobytes()
                    if key not in pats:
                        pats[key] = len(pat_list)
                        pat_list.append(s)
                    mixed.append((c, pats[key]))
            blk.append((jt, off, tuple(mixed)))
        # descending offset so the last block is full width (clean stop)
        blk.sort(key=lambda b: -b[1])
        assert blk and blk[-1][1] == 0, f"ic {ic}: no full-width block"
        blocks_per_ic.append(tuple(blk))
    em_arr = (np.concatenate(pat_list, axis=1) if pat_list
              else np.zeros((128, 128), dtype=np.float32))
    return tuple(blocks_per_ic), max(1, len(pat_list)), em_arr


def kernel(x, Wq, Wk, Wv, Wo, attn_mask):
    x = np.asarray(x, dtype=np.float32)
    Wq = np.asarray(Wq, dtype=np.float32)
    Wk = np.asarray(Wk, dtype=np.float32)
    Wv = np.asarray(Wv, dtype=np.float32)
    Wo = np.asarray(Wo, dtype=np.float32)
    mask = np.asarray(attn_mask, dtype=np.float32).reshape(T, T)

    blocks_per_ic, n_pat, em_arr = _classify(mask)
    scale = np.float32(1.0 / np.sqrt(DH))

    xT = [np.ascontiguousarray(x[b].T).astype(BF) for b in range(B)]
    em_bf = np.ascontiguousarray(em_arr).astype(BF)

    in_maps = []
    for c in range(8):
        b, g = c // 4, c % 4
        rows = slice(E * g, E * (g + 1))
        in_maps.append({
            "xt": xT[b],
            "wq": np.ascontiguousarray((Wq[rows, :] * scale).T).astype(BF),
            "wk": np.ascontiguousarray(Wk[rows, :].T).astype(BF),
            "wv": np.ascontiguousarray(Wv[rows, :].T).astype(BF),
            "wo": np.ascontiguousarray(Wo[:, rows].T).astype(BF),
            "em": em_bf.reshape(128, n_pat, 128),
            "onk": np.ones((128, 1), dtype=BF),
            "onp": np.ones((1, 128), dtype=np.float32),
        })

    global _LAST_IN_MAPS, _LAST_NC
    _LAST_IN_MAPS = in_maps
    key = (blocks_per_ic, n_pat)
    if key not in _NC_CACHE:
        _NC_CACHE[key] = _build(key)
    nc = _NC_CACHE[key]
    _LAST_NC = nc
    res = run_bass_kernel_spmd(nc, in_maps, list(range(8)))
    outs = [np.asarray(r["out"], dtype=np.float32) for r in res.results]
    full = np.stack([
        outs[0] + outs[1] + outs[2] + outs[3],
        outs[4] + outs[5] + outs[6] + outs[7],
    ]).astype(np.float32)
    return full


# revision 4
# speedup vs baseline: 1.3655x; 1.1401x over previous
"""Fused multi-head attention (B=2, T=2048, D=2048, H=16) on 8 trn2 NeuronCores.

Sharding: core c handles batch b=c//4 and heads [4g, 4g+4), g=c%4 (tensor
parallel over heads x data parallel over batch). Each core computes its
4 heads' contribution to out[b] = attn(x[b]) @ Wo^T; the host sums the 4
partials per batch.

v2: single fused loop over 512-token chunks (causality: chunk ic's attention
only needs K/V from chunks <= ic), all-bf16 matmul operands (f32 PSUM),
V projected directly into [token, feature] layout (no PE transposes),
diagonal attention blocks computed at partial width with one shared
128x128 triangular mask constant.

Per chunk ic (tokens [512*ic, 512*ic+512)):
  P1  qT[m][:, chunk] = (Wq_s/sqrt(dh) @ x^T)   per m (4 feature tiles)
      kT[m][:, chunk] =  Wk_s @ x^T
      v[4ic+jl]       =  x-block^T-stationary @ Wv  -> [tok, feat]
  P2  per head h: for each surviving key block jt (descending col offset):
        S^T = kT-block^T-contract @ qT[:, off:]  (PSUM)
        pt  = exp(S^T)  (ACT, bf16)   [triangular sub-block *= tri]
        ctx^T[:, off:] += v-block^T @ pt ; l[off:] += 1^T @ pt
      ctx[h][:, chunk] = cps * broadcast(1/l)
  P3  out[t-block, :] = sum_e ctx^T[e, t-block] @ Wo -> DRAM (f32)
"""

import numpy as np
import ml_dtypes

import concourse.bass as bass
import concourse.mybir as mybir
import concourse.tile as tile
from concourse import bacc
from concourse.bass_utils import run_bass_kernel_spmd

F32 = mybir.dt.float32
F32R = mybir.dt.float32r
BF16 = mybir.dt.bfloat16
EXP = mybir.ActivationFunctionType.Exp
BF = ml_dtypes.bfloat16

B, T, D, H = 2, 2048, 2048, 16
DH = D // H          # 128
E = 512              # features per core (4 heads)
HPC = 4              # heads per core
NT = T // 128        # 16 token tiles
ND = D // 128        # 16 model-dim tiles
NE = E // 128        # 4 e-tiles per core
NI = T // 512        # 4 token chunks
NJ = NT              # 16 key tiles

_NC_CACHE = {}


def _build(blocks_key):
    # blocks_key: tuple over ic of tuple of (jt, off, mixed_tuple) where
    # mixed_tuple is ((c, pat_idx), ...) for 128-col sub-blocks needing an
    # elementwise mask multiply; n_pat = number of distinct mask patterns.
    blocks_per_ic, n_pat = blocks_key
    nc = bacc.Bacc(None, target_bir_lowering=False, debug=False)
    xt = nc.declare_dram_parameter("xt", [D, T], BF16, isOutput=False)
    wq = nc.declare_dram_parameter("wq", [D, E], BF16, isOutput=False)
    wk = nc.declare_dram_parameter("wk", [D, E], BF16, isOutput=False)
    wv = nc.declare_dram_parameter("wv", [D, E], BF16, isOutput=False)
    wo = nc.declare_dram_parameter("wo", [E, D], BF16, isOutput=False)
    em = nc.declare_dram_parameter("em", [128, n_pat, 128], BF16, isOutput=False)
    onk = nc.declare_dram_parameter("onk", [128, 1], BF16, isOutput=False)
    out = nc.declare_dram_parameter("out", [T, D], F32, isOutput=True)

    with tile.TileContext(nc) as tc:
        # ---- long-lived residents ---------------------------------------
        p_res = tc.alloc_tile_pool(name="res", bufs=1)
        qT = [p_res.tile([128, T], BF16, name=f"qT{m}") for m in range(NE)]
        kT = [p_res.tile([128, T], BF16, name=f"kT{m}") for m in range(NE)]
        ctx = [p_res.tile([128, T], BF16, name=f"ctx{m}") for m in range(NE)]
        v_sb = p_res.tile([128, NT, E], BF16)
        wq_sb = p_res.tile([128, ND, E], BF16)
        wk_sb = p_res.tile([128, ND, E], BF16)
        wv_sb = p_res.tile([128, ND, E], BF16)
        wo_sb = p_res.tile([128, NE, D], BF16)
        em_sb = p_res.tile([128, n_pat, 128], BF16)
        onk_sb = p_res.tile([128, 1], BF16)

        for dt in range(ND):
            nc.sync.dma_start(out=wq_sb[:, dt, :], in_=wq.ap()[dt * 128:(dt + 1) * 128, :])
            nc.sync.dma_start(out=wk_sb[:, dt, :], in_=wk.ap()[dt * 128:(dt + 1) * 128, :])
            nc.sync.dma_start(out=wv_sb[:, dt, :], in_=wv.ap()[dt * 128:(dt + 1) * 128, :])
        nc.sync.dma_start(out=em_sb[:, :, :], in_=em.ap())
        nc.sync.dma_start(out=onk_sb, in_=onk.ap())
        for et in range(NE):
            nc.sync.dma_start(out=wo_sb[:, et, :], in_=wo.ap()[et * 128:(et + 1) * 128, :])

        # ---- working pools ----------------------------------------------
        p_x = tc.alloc_tile_pool(name="px", bufs=2)
        p_pt = tc.alloc_tile_pool(name="ppt", bufs=4)
        p_ot = tc.alloc_tile_pool(name="pot", bufs=3)
        p_bs = tc.alloc_tile_pool(name="pbs", bufs=2)
        p_rr = tc.alloc_tile_pool(name="prr", bufs=2)
        ps_big = tc.alloc_tile_pool(name="psbig", bufs=4, space="PSUM")
        ps_cps = tc.alloc_tile_pool(name="pscps", bufs=2, space="PSUM")
        ps_sm = tc.alloc_tile_pool(name="pssm", bufs=1, space="PSUM")

        for ic in range(NI):
            csl = slice(ic * 512, (ic + 1) * 512)
            scope = nc.named_scope(f"chunk{ic}")
            scope.__enter__()

            # ---- P1: projections for this chunk -------------------------
            xc = p_x.tile([128, ND, 512], BF16, name="xc", bufs=2)
            for dt in range(ND):
                nc.sync.dma_start(
                    out=xc[:, dt, :], in_=xt.ap()[dt * 128:(dt + 1) * 128, csl])
            for m in range(NE):
                msl = slice(m * 128, (m + 1) * 128)
                psq = ps_big.tile([128, 512], F32, name="ps", bufs=4)
                psk = ps_big.tile([128, 512], F32, name="ps", bufs=4)
                for dt in range(ND):
                    st, sp = dt == 0, dt == ND - 1
                    nc.tensor.matmul(psq, wq_sb[:, dt, msl], xc[:, dt, :],
                                     start=st, stop=sp)
                    nc.tensor.matmul(psk, wk_sb[:, dt, msl], xc[:, dt, :],
                                     start=st, stop=sp)
                nc.scalar.copy(qT[m][:, csl], psq)
                nc.vector.tensor_copy(kT[m][:, csl], psk)
            for jl in range(4):
                jt = ic * 4 + jl
                psv = ps_big.tile([128, 512], F32, name="ps", bufs=4)
                for dt in range(ND):
                    nc.tensor.matmul(
                        psv, xc[:, dt, jl * 128:(jl + 1) * 128], wv_sb[:, dt, :],
                        start=(dt == 0), stop=(dt == ND - 1))
                nc.vector.tensor_copy(v_sb[:, jt, :], psv)

            # ---- P2: attention for this chunk ---------------------------
            blocks = blocks_per_ic[ic]
            nb = len(blocks)
            for h in range(HPC):
                hsl = slice(h * 128, (h + 1) * 128)
                cps = ps_cps.tile([128, 512], F32, name="cps", bufs=2)
                lps = ps_sm.tile([1, 512], F32, name="lps", bufs=2)
                for bi, (jt, off, mixed) in enumerate(blocks):
                    ps_s = ps_big.tile([128, 512], F32, name="ps", bufs=4)
                    nc.tensor.matmul(
                        ps_s[:, off:512], kT[h][:, jt * 128:(jt + 1) * 128],
                        qT[h][:, ic * 512 + off:(ic + 1) * 512],
                        start=True, stop=True)
                    pt = p_pt.tile([128, 512], BF16, name="pt", bufs=4)
                    nc.scalar.activation(pt[:, off:512], ps_s[:, off:512], EXP)
                    for (c, pidx) in mixed:
                        nc.vector.tensor_mul(
                            pt[:, c * 128:(c + 1) * 128],
                            pt[:, c * 128:(c + 1) * 128],
                            em_sb[:, pidx, :])
                    st, sp = bi == 0, bi == nb - 1
                    nc.tensor.matmul(cps[:, off:512], v_sb[:, jt, hsl],
                                     pt[:, off:512], start=st, stop=sp)
                    nc.tensor.matmul(lps[:, off:512], onk_sb, pt[:, off:512],
                                     start=st, stop=sp)
                rr = p_rr.tile([1, 512], F32, name="rr", bufs=2)
                nc.vector.reciprocal_approx_fast(out=rr, in_=lps)
                rrb = p_bs.tile([128, 512], F32, name="rrb", bufs=2)
                nc.gpsimd.partition_broadcast(rrb, rr)
                nc.vector.tensor_mul(ctx[h][:, csl], cps, rrb)

            # ---- P3: output projection for this chunk's tokens ----------
            for tl in range(4):
                tt = ic * 4 + tl
                tsl = slice(tt * 128, (tt + 1) * 128)
                for nch in range(NI):
                    ps_o = ps_big.tile([128, 512], F32, name="ps", bufs=4)
                    for et in range(NE):
                        nc.tensor.matmul(
                            ps_o, ctx[et][:, tsl],
                            wo_sb[:, et, nch * 512:(nch + 1) * 512],
                            start=(et == 0), stop=(et == NE - 1))
                    ot = p_ot.tile([128, 512], F32, name="ot", bufs=3)
                    if (tl + nch) % 4 == 0:
                        nc.scalar.copy(ot, ps_o)
                    else:
                        nc.vector.tensor_copy(ot, ps_o)
                    nc.sync.dma_start(
                        out=out.ap()[tsl, nch * 512:(nch + 1) * 512], in_=ot)
            scope.__exit__(None, None, None)

        for p in (ps_sm, ps_cps, ps_big, p_rr, p_bs, p_ot, p_pt, p_x, p_res):
            p.release()

    nc.compile()
    return nc


def _classify(mask):
    """Per (ic, jt): column offset + mixed 128-col sub-blocks, from exp(mask)^T."""
    emT = np.ascontiguousarray(np.exp(mask).T)  # [key j, query i]
    pats = {}   # pattern bytes -> index
    pat_list = []
    blocks_per_ic = []
    for ic in range(NI):
        blk = []
        for jt in range(NJ):
            sub = emT[jt * 128:(jt + 1) * 128, ic * 512:(ic + 1) * 512]
            # 128-col sub-block classes
            kinds = []
            for c in range(4):
                s = sub[:, c * 128:(c + 1) * 128]
                if not s.any():
                    kinds.append(0)
                elif np.all(s == 1.0):
                    kinds.append(1)
                else:
                    kinds.append(2)
            if all(k == 0 for k in kinds):
                continue
            first = next(i for i, k in enumerate(kinds) if k != 0)
            off = first * 128
            mixed = []
            for c in range(first, 4):
                if kinds[c] != 1:
                    s = np.asarray(sub[:, c * 128:(c + 1) * 128], dtype=np.float32)
                    key = s.tobytes()
                    if key not in pats:
                        pats[key] = len(pat_list)
                        pat_list.append(s)
                    mixed.append((c, pats[key]))
            blk.append((jt, off, tuple(mixed)))
        # descending offset so the last block is full width (clean stop)
        blk.sort(key=lambda b: -b[1])
        assert blk and blk[-1][1] == 0, f"ic {ic}: no full-width block"
        blocks_per_ic.append(tuple(blk))
    em_arr = (np.concatenate(pat_list, axis=1) if pat_list
              else np.zeros((128, 128), dtype=np.float32))
    return tuple(blocks_per_ic), max(1, len(pat_list)), em_arr


def kernel(x, Wq, Wk, Wv, Wo, attn_mask):
    x = np.asarray(x, dtype=np.float32)
    Wq = np.asarray(Wq, dtype=np.float32)
    Wk = np.asarray(Wk, dtype=np.float32)
    Wv = np.asarray(Wv, dtype=np.float32)
    Wo = np.asarray(Wo, dtype=np.float32)
    mask = np.asarray(attn_mask, dtype=np.float32).reshape(T, T)

    blocks_per_ic, n_pat, em_arr = _classify(mask)
    scale = np.float32(1.0 / np.sqrt(DH))

    xT = [np.ascontiguousarray(x[b].T).astype(BF) for b in range(B)]
    em_bf = np.ascontiguousarray(em_arr).astype(BF)

    in_maps = []
    for c in range(8):
        b, g = c // 4, c % 4
        rows = slice(E * g, E * (g + 1))
        in_maps.append({
            "xt": xT[b],
            "wq": np.ascontiguousarray((Wq[rows, :] * scale).T).astype(BF),
            "wk": np.ascontiguousarray(Wk[rows, :].T).astype(BF),
            "wv": np.ascontiguousarray(Wv[rows, :].T).astype(BF),
            "wo": np.ascontiguousarray(Wo[:, rows].T).astype(BF),
            "em": em_bf.reshape(128, n_pat, 128),
            "onk": np.ones((128, 1), dtype=BF),
        })

    global _LAST_IN_MAPS, _LAST_NC
    _LAST_IN_MAPS = in_maps
    key = (blocks_per_ic, n_pat)
    if key not in _NC_CACHE:
        _NC_CACHE[key] = _build(key)
    nc = _NC_CACHE[key]
    _LAST_NC = nc
    res = run_bass_kernel_spmd(nc, in_maps, list(range(8)))
    outs = [np.asarray(r["out"], dtype=np.float32) for r in res.results]
    full = np.stack([
        outs[0] + outs[1] + outs[2] + outs[3],
        outs[4] + outs[5] + outs[6] + outs[7],
    ]).astype(np.float32)
    return full


# revision 5
# speedup vs baseline: 1.4295x; 1.0469x over previous
"""Fused multi-head attention (B=2, T=2048, D=2048, H=16) on 8 trn2 NeuronCores.

Sharding: core c handles batch b=c//4 and heads [4g, 4g+4), g=c%4 (tensor
parallel over heads x data parallel over batch). Each core computes its
4 heads' contribution to out[b] = attn(x[b]) @ Wo^T; the host sums the 4
partials per batch.

v2: single fused loop over 512-token chunks (causality: chunk ic's attention
only needs K/V from chunks <= ic), all-bf16 matmul operands (f32 PSUM),
V projected directly into [token, feature] layout (no PE transposes),
diagonal attention blocks computed at partial width with one shared
128x128 triangular mask constant.

Per chunk ic (tokens [512*ic, 512*ic+512)):
  P1  qT[m][:, chunk] = (Wq_s/sqrt(dh) @ x^T)   per m (4 feature tiles)
      kT[m][:, chunk] =  Wk_s @ x^T
      v[4ic+jl]       =  x-block^T-stationary @ Wv  -> [tok, feat]
  P2  per head h: for each surviving key block jt (descending col offset):
        S^T = kT-block^T-contract @ qT[:, off:]  (PSUM)
        pt  = exp(S^T)  (ACT, bf16)   [triangular sub-block *= tri]
        ctx^T[:, off:] += v-block^T @ pt ; l[off:] += 1^T @ pt
      ctx[h][:, chunk] = cps * broadcast(1/l)
  P3  out[t-block, :] = sum_e ctx^T[e, t-block] @ Wo -> DRAM (f32)
"""

import numpy as np
import ml_dtypes

import concourse.bass as bass
import concourse.mybir as mybir
import concourse.tile as tile
from concourse import bacc
from concourse.bass_utils import run_bass_kernel_spmd

F32 = mybir.dt.float32
F32R = mybir.dt.float32r
BF16 = mybir.dt.bfloat16
EXP = mybir.ActivationFunctionType.Exp
BF = ml_dtypes.bfloat16

B, T, D, H = 2, 2048, 2048, 16
DH = D // H          # 128
E = 512              # features per core (4 heads)
HPC = 4              # heads per core
NT = T // 128        # 16 token tiles
ND = D // 128        # 16 model-dim tiles
NE = E // 128        # 4 e-tiles per core
NI = T // 512        # 4 token chunks
NJ = NT              # 16 key tiles

_NC_CACHE = {}


def _build(blocks_key):
    # blocks_key: tuple over ic of tuple of (jt, off, mixed_tuple) where
    # mixed_tuple is ((c, pat_idx), ...) for 128-col sub-blocks needing an
    # elementwise mask multiply; n_pat = number of distinct mask patterns.
    blocks_per_ic, n_pat = blocks_key
    nc = bacc.Bacc(None, target_bir_lowering=False, debug=False)
    xt = nc.declare_dram_parameter("xt", [D, T], BF16, isOutput=False)
    wq = nc.declare_dram_parameter("wq", [D, E], BF16, isOutput=False)
    wk = nc.declare_dram_parameter("wk", [D, E], BF16, isOutput=False)
    wv = nc.declare_dram_parameter("wv", [D, E], BF16, isOutput=False)
    wo = nc.declare_dram_parameter("wo", [E, D], BF16, isOutput=False)
    em = nc.declare_dram_parameter("em", [128, n_pat, 128], BF16, isOutput=False)
    onk = nc.declare_dram_parameter("onk", [128, 1], BF16, isOutput=False)
    out = nc.declare_dram_parameter("out", [T, D], F32, isOutput=True)

    with tile.TileContext(nc) as tc:
        # ---- long-lived residents ---------------------------------------
        p_res = tc.alloc_tile_pool(name="res", bufs=1)
        qT = [p_res.tile([128, T], BF16, name=f"qT{m}") for m in range(NE)]
        kT = [p_res.tile([128, T], BF16, name=f"kT{m}") for m in range(NE)]
        ctx = [p_res.tile([128, T], BF16, name=f"ctx{m}") for m in range(NE)]
        v_sb = p_res.tile([128, NT, E], BF16)
        wq_sb = p_res.tile([128, ND, E], BF16)
        wk_sb = p_res.tile([128, ND, E], BF16)
        wv_sb = p_res.tile([128, ND, E], BF16)
        wo_sb = p_res.tile([128, NE, D], BF16)
        em_sb = p_res.tile([128, n_pat, 128], BF16)
        onk_sb = p_res.tile([128, 1], BF16)

        # ---- working pools ----------------------------------------------
        p_x = tc.alloc_tile_pool(name="px", bufs=2)
        p_pt = tc.alloc_tile_pool(name="ppt", bufs=4)
        p_ot = tc.alloc_tile_pool(name="pot", bufs=3)
        p_bs = tc.alloc_tile_pool(name="pbs", bufs=2)
        p_rr = tc.alloc_tile_pool(name="prr", bufs=2)
        ps_big = tc.alloc_tile_pool(name="psbig", bufs=4, space="PSUM")
        ps_cps = tc.alloc_tile_pool(name="pscps", bufs=2, space="PSUM")
        ps_sm = tc.alloc_tile_pool(name="pssm", bufs=1, space="PSUM")

        # DMA emission in first-use order: chunk-0 x interleaved with q/k
        # weights (P1 needs both immediately), then v weights, mask consts
        # (P2), and wo last (first P3 is ~80us in).
        xc0 = p_x.tile([128, ND, 512], BF16, name="xc", bufs=2)
        for dt in range(ND):
            nc.sync.dma_start(
                out=xc0[:, dt, :], in_=xt.ap()[dt * 128:(dt + 1) * 128, 0:512])
            nc.sync.dma_start(out=wq_sb[:, dt, :], in_=wq.ap()[dt * 128:(dt + 1) * 128, :])
            nc.sync.dma_start(out=wk_sb[:, dt, :], in_=wk.ap()[dt * 128:(dt + 1) * 128, :])
        for dt in range(ND):
            nc.sync.dma_start(out=wv_sb[:, dt, :], in_=wv.ap()[dt * 128:(dt + 1) * 128, :])
        nc.sync.dma_start(out=em_sb[:, :, :], in_=em.ap())
        nc.sync.dma_start(out=onk_sb, in_=onk.ap())
        for et in range(NE):
            nc.sync.dma_start(out=wo_sb[:, et, :], in_=wo.ap()[et * 128:(et + 1) * 128, :])

        for ic in range(NI):
            csl = slice(ic * 512, (ic + 1) * 512)
            scope = nc.named_scope(f"chunk{ic}")
            scope.__enter__()

            # ---- P1: projections for this chunk -------------------------
            if ic == 0:
                xc = xc0
            else:
                xc = p_x.tile([128, ND, 512], BF16, name="xc", bufs=2)
                for dt in range(ND):
                    nc.sync.dma_start(
                        out=xc[:, dt, :], in_=xt.ap()[dt * 128:(dt + 1) * 128, csl])
            for m in range(NE):
                msl = slice(m * 128, (m + 1) * 128)
                psq = ps_big.tile([128, 512], F32, name="ps", bufs=4)
                psk = ps_big.tile([128, 512], F32, name="ps", bufs=4)
                for dt in range(ND):
                    st, sp = dt == 0, dt == ND - 1
                    nc.tensor.matmul(psq, wq_sb[:, dt, msl], xc[:, dt, :],
                                     start=st, stop=sp)
                    nc.tensor.matmul(psk, wk_sb[:, dt, msl], xc[:, dt, :],
                                     start=st, stop=sp)
                nc.scalar.copy(qT[m][:, csl], psq)
                nc.vector.tensor_copy(kT[m][:, csl], psk)
            for jl in range(4):
                jt = ic * 4 + jl
                psv = ps_big.tile([128, 512], F32, name="ps", bufs=4)
                for dt in range(ND):
                    nc.tensor.matmul(
                        psv, xc[:, dt, jl * 128:(jl + 1) * 128], wv_sb[:, dt, :],
                        start=(dt == 0), stop=(dt == ND - 1))
                nc.vector.tensor_copy(v_sb[:, jt, :], psv)

            # ---- P2: attention for this chunk ---------------------------
            blocks = blocks_per_ic[ic]
            nb = len(blocks)
            for h in range(HPC):
                hsl = slice(h * 128, (h + 1) * 128)
                cps = ps_cps.tile([128, 512], F32, name="cps", bufs=2)
                lps = ps_sm.tile([1, 512], F32, name="lps", bufs=2)
                for bi, (jt, off, mixed) in enumerate(blocks):
                    ps_s = ps_big.tile([128, 512], F32, name="ps", bufs=4)
                    nc.tensor.matmul(
                        ps_s[:, off:512], kT[h][:, jt * 128:(jt + 1) * 128],
                        qT[h][:, ic * 512 + off:(ic + 1) * 512],
                        start=True, stop=True)
                    pt = p_pt.tile([128, 512], BF16, name="pt", bufs=4)
                    nc.scalar.activation(pt[:, off:512], ps_s[:, off:512], EXP)
                    for (c, pidx) in mixed:
                        nc.vector.tensor_mul(
                            pt[:, c * 128:(c + 1) * 128],
                            pt[:, c * 128:(c + 1) * 128],
                            em_sb[:, pidx, :])
                    st, sp = bi == 0, bi == nb - 1
                    nc.tensor.matmul(cps[:, off:512], v_sb[:, jt, hsl],
                                     pt[:, off:512], start=st, stop=sp)
                    nc.tensor.matmul(lps[:, off:512], onk_sb, pt[:, off:512],
                                     start=st, stop=sp)
                rr = p_rr.tile([1, 512], F32, name="rr", bufs=2)
                nc.vector.reciprocal_approx_fast(out=rr, in_=lps)
                rrb = p_bs.tile([128, 512], F32, name="rrb", bufs=2)
                nc.gpsimd.partition_broadcast(rrb, rr)
                nc.vector.tensor_mul(ctx[h][:, csl], cps, rrb)

            # ---- P3: output projection for this chunk's tokens ----------
            for tl in range(4):
                tt = ic * 4 + tl
                tsl = slice(tt * 128, (tt + 1) * 128)
                for nch in range(NI):
                    ps_o = ps_big.tile([128, 512], F32, name="ps", bufs=4)
                    for et in range(NE):
                        nc.tensor.matmul(
                            ps_o, ctx[et][:, tsl],
                            wo_sb[:, et, nch * 512:(nch + 1) * 512],
                            start=(et == 0), stop=(et == NE - 1))
                    ot = p_ot.tile([128, 512], F32, name="ot", bufs=3)
                    if (tl + nch) % 4 == 0:
                        nc.scalar.copy(ot, ps_o)
                    else:
                        nc.vector.tensor_copy(ot, ps_o)
                    nc.sync.dma_start(
                        out=out.ap()[tsl, nch * 512:(nch + 1) * 512], in_=ot)
            scope.__exit__(None, None, None)

        for p in (ps_sm, ps_cps, ps_big, p_rr, p_bs, p_ot, p_pt, p_x, p_res):
            p.release()

    nc.compile()
    return nc


def _classify(mask):
    """Per (ic, jt): column offset + mixed 128-col sub-blocks, from exp(mask)^T."""
    emT = np.ascontiguousarray(np.exp(mask).T)  # [key j, query i]
    pats = {}   # pattern bytes -> index
    pat_list = []
    blocks_per_ic = []
    for ic in range(NI):
        blk = []
        for jt in range(NJ):
            sub = emT[jt * 128:(jt + 1) * 128, ic * 512:(ic + 1) * 512]
            # 128-col sub-block classes
            kinds = []
            for c in range(4):
                s = sub[:, c * 128:(c + 1) * 128]
                if not s.any():
                    kinds.append(0)
                elif np.all(s == 1.0):
                    kinds.append(1)
                else:
                    kinds.append(2)
            if all(k == 0 for k in kinds):
                continue
            first = next(i for i, k in enumerate(kinds) if k != 0)
            off = first * 128
            mixed = []
            for c in range(first, 4):
                if kinds[c] != 1:
                    s = np.asarray(sub[:, c * 128:(c + 1) * 128], dtype=np.float32)
                    key = s.tobytes()
                    if key not in pats:
                        pats[key] = len(pat_list)
                        pat_list.append(s)
                    mixed.append((c, pats[key]))
            blk.append((jt, off, tuple(mixed)))
        # descending offset so the last block is full width (clean stop)
        blk.sort(key=lambda b: -b[1])
        assert blk and blk[-1][1] == 0, f"ic {ic}: no full-width block"
        blocks_per_ic.append(tuple(blk))
    em_arr = (np.concatenate(pat_list, axis=1) if pat_list
              else np.zeros((128, 128), dtype=np.float32))
    return tuple(blocks_per_ic), max(1, len(pat_list)), em_arr


def kernel(x, Wq, Wk, Wv, Wo, attn_mask):
    x = np.asarray(x, dtype=np.float32)
    Wq = np.asarray(Wq, dtype=np.float32)
    Wk = np.asarray(Wk, dtype=np.float32)
    Wv = np.asarray(Wv, dtype=np.float32)
    Wo = np.asarray(Wo, dtype=np.float32)
    mask = np.asarray(attn_mask, dtype=np.float32).reshape(T, T)

    blocks_per_ic, n_pat, em_arr = _classify(mask)
    scale = np.float32(1.0 / np.sqrt(DH))

    xT = [np.ascontiguousarray(x[b].T).astype(BF) for b in range(B)]
    em_bf = np.ascontiguousarray(em_arr).astype(BF)

    in_maps = []
    for c in range(8):
        b, g = c // 4, c % 4
        rows = slice(E * g, E * (g + 1))
        in_maps.append({
            "xt": xT[b],
            "wq": np.ascontiguousarray((Wq[rows, :] * scale).T).astype(BF),
            "wk": np.ascontiguousarray(Wk[rows, :].T).astype(BF),
            "wv": np.ascontiguousarray(Wv[rows, :].T).astype(BF),
            "wo": np.ascontiguousarray(Wo[:, rows].T).astype(BF),
            "em": em_bf.reshape(128, n_pat, 128),
            "onk": np.ones((128, 1), dtype=BF),
        })

    global _LAST_IN_MAPS, _LAST_NC
    _LAST_IN_MAPS = in_maps
    key = (blocks_per_ic, n_pat)
    if key not in _NC_CACHE:
        _NC_CACHE[key] = _build(key)
    nc = _NC_CACHE[key]
    _LAST_NC = nc
    res = run_bass_kernel_spmd(nc, in_maps, list(range(8)))
    outs = [np.asarray(r["out"], dtype=np.float32) for r in res.results]
    full = np.stack([
        outs[0] + outs[1] + outs[2] + outs[3],
        outs[4] + outs[5] + outs[6] + outs[7],
    ]).astype(np.float32)
    return full


# revision 6
# speedup vs baseline: 1.4472x; 1.0124x over previous
"""Fused multi-head attention (B=2, T=2048, D=2048, H=16) on 8 trn2 NeuronCores.

Sharding: core c handles batch b=c//4 and heads [4g, 4g+4), g=c%4 (tensor
parallel over heads x data parallel over batch). Each core computes its
4 heads' contribution to out[b] = attn(x[b]) @ Wo^T; the host sums the 4
partials per batch.

v2: single fused loop over 512-token chunks (causality: chunk ic's attention
only needs K/V from chunks <= ic), all-bf16 matmul operands (f32 PSUM),
V projected directly into [token, feature] layout (no PE transposes),
diagonal attention blocks computed at partial width with one shared
128x128 triangular mask constant.

Per chunk ic (tokens [512*ic, 512*ic+512)):
  P1  qT[m][:, chunk] = (Wq_s/sqrt(dh) @ x^T)   per m (4 feature tiles)
      kT[m][:, chunk] =  Wk_s @ x^T
      v[4ic+jl]       =  x-block^T-stationary @ Wv  -> [tok, feat]
  P2  per head h: for each surviving key block jt (descending col offset):
        S^T = kT-block^T-contract @ qT[:, off:]  (PSUM)
        pt  = exp(S^T)  (ACT, bf16)   [triangular sub-block *= tri]
        ctx^T[:, off:] += v-block^T @ pt ; l[off:] += 1^T @ pt
      ctx[h][:, chunk] = cps * broadcast(1/l)
  P3  out[t-block, :] = sum_e ctx^T[e, t-block] @ Wo -> DRAM (f32)
"""

import numpy as np
import ml_dtypes

import concourse.bass as bass
import concourse.mybir as mybir
import concourse.tile as tile
from concourse import bacc
from concourse.bass_utils import run_bass_kernel_spmd

F32 = mybir.dt.float32
F32R = mybir.dt.float32r
BF16 = mybir.dt.bfloat16
EXP = mybir.ActivationFunctionType.Exp
BF = ml_dtypes.bfloat16

B, T, D, H = 2, 2048, 2048, 16
DH = D // H          # 128
E = 512              # features per core (4 heads)
HPC = 4              # heads per core
NT = T // 128        # 16 token tiles
ND = D // 128        # 16 model-dim tiles
NE = E // 128        # 4 e-tiles per core
NI = T // 512        # 4 token chunks
NJ = NT              # 16 key tiles

_NC_CACHE = {}


def _build(blocks_key):
    # blocks_key: tuple over ic of tuple of (jt, off, mixed_tuple) where
    # mixed_tuple is ((c, pat_idx), ...) for 128-col sub-blocks needing an
    # elementwise mask multiply; n_pat = number of distinct mask patterns.
    blocks_per_ic, n_pat = blocks_key
    nc = bacc.Bacc(None, target_bir_lowering=False, debug=False)
    xt = nc.declare_dram_parameter("xt", [D, T], BF16, isOutput=False)
    wq = nc.declare_dram_parameter("wq", [D, E], BF16, isOutput=False)
    wk = nc.declare_dram_parameter("wk", [D, E], BF16, isOutput=False)
    wv = nc.declare_dram_parameter("wv", [D, E], BF16, isOutput=False)
    wo = nc.declare_dram_parameter("wo", [E, D], BF16, isOutput=False)
    em = nc.declare_dram_parameter("em", [128, n_pat, 128], BF16, isOutput=False)
    onk = nc.declare_dram_parameter("onk", [128, 1], BF16, isOutput=False)
    out = nc.declare_dram_parameter("out", [T, D], F32, isOutput=True)

    with tile.TileContext(nc) as tc:
        # ---- long-lived residents ---------------------------------------
        p_res = tc.alloc_tile_pool(name="res", bufs=1)
        qT = [p_res.tile([128, T], BF16, name=f"qT{m}") for m in range(NE)]
        kT = [p_res.tile([128, T], BF16, name=f"kT{m}") for m in range(NE)]
        ctx = [p_res.tile([128, T], BF16, name=f"ctx{m}") for m in range(NE)]
        v_sb = p_res.tile([128, NT, E], BF16)
        wq_sb = p_res.tile([128, ND, E], BF16)
        wk_sb = p_res.tile([128, ND, E], BF16)
        wv_sb = p_res.tile([128, ND, E], BF16)
        wo_sb = p_res.tile([128, NE, D], BF16)
        em_sb = p_res.tile([128, n_pat, 128], BF16)
        onk_sb = p_res.tile([128, 1], BF16)

        # ---- working pools ----------------------------------------------
        p_x = tc.alloc_tile_pool(name="px", bufs=2)
        p_pt = tc.alloc_tile_pool(name="ppt", bufs=4)
        p_ot = tc.alloc_tile_pool(name="pot", bufs=3)
        p_bs = tc.alloc_tile_pool(name="pbs", bufs=2)
        p_rr = tc.alloc_tile_pool(name="prr", bufs=2)
        ps_big = tc.alloc_tile_pool(name="psbig", bufs=4, space="PSUM")
        ps_cps = tc.alloc_tile_pool(name="pscps", bufs=2, space="PSUM")
        ps_sm = tc.alloc_tile_pool(name="pssm", bufs=1, space="PSUM")

        # DMA emission in first-use order: chunk-0 x interleaved with q/k
        # weights (P1 needs both immediately), then v weights, mask consts
        # (P2), and wo last (first P3 is ~80us in).
        xcs = {}
        xcs[0] = p_x.tile([128, ND, 512], BF16, name="xc", bufs=2)
        for dt in range(ND):
            nc.sync.dma_start(
                out=xcs[0][:, dt, :], in_=xt.ap()[dt * 128:(dt + 1) * 128, 0:512])
            nc.sync.dma_start(out=wq_sb[:, dt, :], in_=wq.ap()[dt * 128:(dt + 1) * 128, :])
            nc.sync.dma_start(out=wk_sb[:, dt, :], in_=wk.ap()[dt * 128:(dt + 1) * 128, :])
        for dt in range(ND):
            nc.sync.dma_start(out=wv_sb[:, dt, :], in_=wv.ap()[dt * 128:(dt + 1) * 128, :])
        nc.sync.dma_start(out=em_sb[:, :, :], in_=em.ap())
        nc.sync.dma_start(out=onk_sb, in_=onk.ap())
        for et in range(NE):
            nc.sync.dma_start(out=wo_sb[:, et, :], in_=wo.ap()[et * 128:(et + 1) * 128, :])

        for ic in range(NI):
            csl = slice(ic * 512, (ic + 1) * 512)
            scope = nc.named_scope(f"chunk{ic}")
            scope.__enter__()

            # ---- P1: projections for this chunk -------------------------
            # prefetch next chunk's x ahead of this chunk's output stores
            if ic + 1 < NI:
                nsl = slice((ic + 1) * 512, (ic + 2) * 512)
                xcs[ic + 1] = p_x.tile([128, ND, 512], BF16, name="xc", bufs=2)
                for dt in range(ND):
                    nc.sync.dma_start(
                        out=xcs[ic + 1][:, dt, :],
                        in_=xt.ap()[dt * 128:(dt + 1) * 128, nsl])
            xc = xcs.pop(ic)
            for m in range(NE):
                msl = slice(m * 128, (m + 1) * 128)
                psq = ps_big.tile([128, 512], F32, name="ps", bufs=4)
                psk = ps_big.tile([128, 512], F32, name="ps", bufs=4)
                if ic == 0 and m == 0:
                    # start of kernel is DMA-bound: q sweep first (needs only
                    # xc+wq), k sweep second while wk still streams in
                    for dt in range(ND):
                        nc.tensor.matmul(psq, wq_sb[:, dt, msl], xc[:, dt, :],
                                         start=dt == 0, stop=dt == ND - 1)
                    for dt in range(ND):
                        nc.tensor.matmul(psk, wk_sb[:, dt, msl], xc[:, dt, :],
                                         start=dt == 0, stop=dt == ND - 1)
                else:
                    for dt in range(ND):
                        st, sp = dt == 0, dt == ND - 1
                        nc.tensor.matmul(psq, wq_sb[:, dt, msl], xc[:, dt, :],
                                         start=st, stop=sp)
                        nc.tensor.matmul(psk, wk_sb[:, dt, msl], xc[:, dt, :],
                                         start=st, stop=sp)
                nc.scalar.copy(qT[m][:, csl], psq)
                nc.vector.tensor_copy(kT[m][:, csl], psk)
            for jl in range(4):
                jt = ic * 4 + jl
                psv = ps_big.tile([128, 512], F32, name="ps", bufs=4)
                for dt in range(ND):
                    nc.tensor.matmul(
                        psv, xc[:, dt, jl * 128:(jl + 1) * 128], wv_sb[:, dt, :],
                        start=(dt == 0), stop=(dt == ND - 1))
                nc.vector.tensor_copy(v_sb[:, jt, :], psv)

            # ---- P2: attention for this chunk ---------------------------
            blocks = blocks_per_ic[ic]
            nb = len(blocks)
            for h in range(HPC):
                hsl = slice(h * 128, (h + 1) * 128)
                cps = ps_cps.tile([128, 512], F32, name="cps", bufs=2)
                lps = ps_sm.tile([1, 512], F32, name="lps", bufs=2)
                for bi, (jt, off, mixed) in enumerate(blocks):
                    ps_s = ps_big.tile([128, 512], F32, name="ps", bufs=4)
                    nc.tensor.matmul(
                        ps_s[:, off:512], kT[h][:, jt * 128:(jt + 1) * 128],
                        qT[h][:, ic * 512 + off:(ic + 1) * 512],
                        start=True, stop=True)
                    pt = p_pt.tile([128, 512], BF16, name="pt", bufs=4)
                    nc.scalar.activation(pt[:, off:512], ps_s[:, off:512], EXP)
                    for (c, pidx) in mixed:
                        nc.vector.tensor_mul(
                            pt[:, c * 128:(c + 1) * 128],
                            pt[:, c * 128:(c + 1) * 128],
                            em_sb[:, pidx, :])
                    st, sp = bi == 0, bi == nb - 1
                    nc.tensor.matmul(cps[:, off:512], v_sb[:, jt, hsl],
                                     pt[:, off:512], start=st, stop=sp)
                    nc.tensor.matmul(lps[:, off:512], onk_sb, pt[:, off:512],
                                     start=st, stop=sp)
                rr = p_rr.tile([1, 512], F32, name="rr", bufs=2)
                nc.vector.reciprocal_approx_fast(out=rr, in_=lps)
                rrb = p_bs.tile([128, 512], F32, name="rrb", bufs=2)
                nc.gpsimd.partition_broadcast(rrb, rr)
                nc.vector.tensor_mul(ctx[h][:, csl], cps, rrb)

            # ---- P3: output projection for this chunk's tokens ----------
            for tl in range(4):
                tt = ic * 4 + tl
                tsl = slice(tt * 128, (tt + 1) * 128)
                for nch in range(NI):
                    ps_o = ps_big.tile([128, 512], F32, name="ps", bufs=4)
                    for et in range(NE):
                        nc.tensor.matmul(
                            ps_o, ctx[et][:, tsl],
                            wo_sb[:, et, nch * 512:(nch + 1) * 512],
                            start=(et == 0), stop=(et == NE - 1))
                    ot = p_ot.tile([128, 512], F32, name="ot", bufs=3)
                    if (tl + nch) % 4 == 0:
                        nc.scalar.copy(ot, ps_o)
                    else:
                        nc.vector.tensor_copy(ot, ps_o)
                    nc.sync.dma_start(
                        out=out.ap()[tsl, nch * 512:(nch + 1) * 512], in_=ot)
            scope.__exit__(None, None, None)

        for p in (ps_sm, ps_cps, ps_big, p_rr, p_bs, p_ot, p_pt, p_x, p_res):
            p.release()

    nc.compile()
    return nc


def _classify(mask):
    """Per (ic, jt): column offset + mixed 128-col sub-blocks, from exp(mask)^T."""
    emT = np.ascontiguousarray(np.exp(mask).T)  # [key j, query i]
    pats = {}   # pattern bytes -> index
    pat_list = []
    blocks_per_ic = []
    for ic in range(NI):
        blk = []
        for jt in range(NJ):
            sub = emT[jt * 128:(jt + 1) * 128, ic * 512:(ic + 1) * 512]
            # 128-col sub-block classes
            kinds = []
            for c in range(4):
                s = sub[:, c * 128:(c + 1) * 128]
                if not s.any():
                    kinds.append(0)
                elif np.all(s == 1.0):
                    kinds.append(1)
                else:
                    kinds.append(2)
            if all(k == 0 for k in kinds):
                continue
            first = next(i for i, k in enumerate(kinds) if k != 0)
            off = first * 128
            mixed = []
            for c in range(first, 4):
                if kinds[c] != 1:
                    s = np.asarray(sub[:, c * 128:(c + 1) * 128], dtype=np.float32)
                    key = s.tobytes()
                    if key not in pats:
                        pats[key] = len(pat_list)
                        pat_list.append(s)
                    mixed.append((c, pats[key]))
            blk.append((jt, off, tuple(mixed)))
        # descending offset so the last block is full width (clean stop)
        blk.sort(key=lambda b: -b[1])
        assert blk and blk[-1][1] == 0, f"ic {ic}: no full-width block"
        blocks_per_ic.append(tuple(blk))
    em_arr = (np.concatenate(pat_list, axis=1) if pat_list
              else np.zeros((128, 128), dtype=np.float32))
    return tuple(blocks_per_ic), max(1, len(pat_list)), em_arr


def kernel(x, Wq, Wk, Wv, Wo, attn_mask):
    x = np.asarray(x, dtype=np.float32)
    Wq = np.asarray(Wq, dtype=np.float32)
    Wk = np.asarray(Wk, dtype=np.float32)
    Wv = np.asarray(Wv, dtype=np.float32)
    Wo = np.asarray(Wo, dtype=np.float32)
    mask = np.asarray(attn_mask, dtype=np.float32).reshape(T, T)

    blocks_per_ic, n_pat, em_arr = _classify(mask)
    scale = np.float32(1.0 / np.sqrt(DH))

    xT = [np.ascontiguousarray(x[b].T).astype(BF) for b in range(B)]
    em_bf = np.ascontiguousarray(em_arr).astype(BF)

    in_maps = []
    for c in range(8):
        b, g = c // 4, c % 4
        rows = slice(E * g, E * (g + 1))
        in_maps.append({
            "xt": xT[b],
            "wq": np.ascontiguousarray((Wq[rows, :] * scale).T).astype(BF),
            "wk": np.ascontiguousarray(Wk[rows, :].T).astype(BF),
            "wv": np.ascontiguousarray(Wv[rows, :].T).astype(BF),
            "wo": np.ascontiguousarray(Wo[:, rows].T).astype(BF),
            "em": em_bf.reshape(128, n_pat, 128),
            "onk": np.ones((128, 1), dtype=BF),
        })

    global _LAST_IN_MAPS, _LAST_NC
    _LAST_IN_MAPS = in_maps
    key = (blocks_per_ic, n_pat)
    if key not in _NC_CACHE:
        _NC_CACHE[key] = _build(key)
    nc = _NC_CACHE[key]
    _LAST_NC = nc
    res = run_bass_kernel_spmd(nc, in_maps, list(range(8)))
    outs = [np.asarray(r["out"], dtype=np.float32) for r in res.results]
    full = np.stack([
        outs[0] + outs[1] + outs[2] + outs[3],
        outs[4] + outs[5] + outs[6] + outs[7],
    ]).astype(np.float32)
    return full
